# revision 33
# baseline (speedup 1.0000x reference)
"""CascadeAttention TRN2 kernel — 8-core head-sharded tensor parallel.

Sharding: each of the 8 NeuronCores owns 4 query heads + 1 KV head (GQA group).
Per core: qkv projections (fp32r matmuls), RoPE, cascade attention over
(sink + window + current) keys with causal masking on the current block,
softmax without max-subtraction (scores are small), o_proj partial product.
Host: shards weights/caches per head, precomputes cos/sin position tables
(ACT Sin has no range reduction on large angles), transposes hidden_states
once (activations must enter the PE contraction-dim-major), and sums the 8
o_proj partials.

All matmuls run in float32r (TF32-like, 1 cycle/row at N>=256 vs 4 for fp32;
measured relerr 1.6e-4) — inputs are either DMA'd into float32r-declared
tensors or written as float32r by the producing compute op, which is what the
walrus FP32r verifier requires.

Schedule notes (v2):
- Weight DMAs are per-dt tiles interleaved with qb0's hst stream so the first
  projection matmul starts ~2us in instead of waiting 40us for bulk weights.
- Cache K/V load + rope (A2) is emitted mid-phase-A so it overlaps the
  projection matmuls instead of serializing between phases A and B.
- Softmax denominator accumulation is off the PE: cache-tile ex sums run as
  DVE/Pool(gpsimd) tensor_add chains into SBUF accumulators; the PE only
  folds the accumulators (ones^T @ acc) at group end. This turns phase B
  from PE-bound into ACT(exp)-bound.
- o_proj partials are written as bf16 (host upconverts and sums), halving
  phase-C write traffic.
"""
import os
import sys

for _p in ("/root/.axon_site/_ro/trn_rl_repo", "/opt/trn_rl_repo"):
    if os.path.isdir(_p) and _p not in sys.path:
        sys.path.insert(0, _p)

import ml_dtypes
import numpy as np

import concourse.bass as bass
import concourse.mybir as mybir
import concourse.tile as tile
from concourse.bass_utils import run_bass_kernel_spmd
from concourse.vector_clock import ScopedClock, VectorClock

F32 = mybir.dt.float32
F32R = mybir.dt.float32r
BF16 = mybir.dt.bfloat16
AF = mybir.ActivationFunctionType

B, Q, D = 1, 2048, 4096
H, KVH, HD = 32, 8, 128
NS, NW = 4, 2048
G = H // KVH           # q heads per kv head = heads per core
NC_CORES = 8
ROPE_BASE = 10000.0

QB = 512               # q block (matmul moving dim)
NQB = Q // QB          # 4
NDT = D // 128         # 32 contraction tiles
NKC = 17               # cache key tiles: 4 sink + 2048 window + 124 pad = 2176
KC = NKC * 128         # 2176
SCALE = 1.0 / float(np.sqrt(HD))
NEG = float(np.finfo(np.float32).min)

# den accumulation split: cache kt in [0,2) init the DVE acc pair; cache kt in
# [2, 2+2*POOL_DEN) go to the Pool(gpsimd) acc pair; the rest of the cache kts
# and all current kts run on the DVE pair.
POOL_DEN = 4           # cache kts per acc on the Pool pair (2 accs -> 8 kts)


# ---------------------------------------------------------------------------
# TileContext tail-drain patch: stock _drain_and_barrier puts one sync-wait per
# outstanding processor on a single SP Drain, overflowing walrus's per-
# instruction wait slots. Split the waits across per-proc SP NoOps instead.
def _split_drain_and_barrier(self, tick_clock, wait_clock):
    nc = self.nc
    gc = tick_clock.global_clock
    n = len(gc)
    for i in range(n):
        t = gc[i]
        if t > 0:
            vec = [0] * n
            vec[i] = t
            nop = nc.sync.nop(nofuse=True, hint=f"tail_wait_p{i}")
            wait_clock.add_sem_waits(nop.ins, ScopedClock({None: VectorClock(vec)}))
    drain_inst = nc.sync.drain()
    full = ScopedClock({None: tick_clock.global_clock})
    wait_clock.add_sem_waits(drain_inst.ins, full, full.copy())
    nc.all_engine_barrier()
    assert self.sems is not None
    popped = nc._tile_sem_poison_stack.pop()
    assert popped is self._sem_poison
    nc.clear_and_free_semaphores(list(self.sems.allocated().values()))
    nc.all_engine_barrier()


tile.TileContext._drain_and_barrier = _split_drain_and_barrier


def _split_excess_waits(nc, cap=1):
    """Walrus enforces small per-instruction sync-wait limits (1-2 depending
    on the lowered encoding). Tile emits up to ~4 on body instructions and
    more on drains. Move excess waits onto same-engine NoOps placed directly
    before the instruction — sems are monotonic in the kernel body, so
    waiting earlier on the same engine is semantically identical."""
    import bass_rust as _br
    for f in nc.m.functions:
        for bb in f.blocks:
            il = bb.instructions
            out = []
            changed = False
            for inst in il:
                si = inst.sync_info
                waits = list(si.on_wait) if (si is not None and si.on_wait) else []
                if len(waits) > cap:
                    changed = True
                    for j, w in enumerate(waits[:-cap]):
                        nop = mybir.InstNoOp(
                            name=f"{inst.name}-w{j}", ins=[], outs=[])
                        nop.engine = inst.engine
                        nop.sync_info = _br.SyncInfo(on_wait=[w], on_update=[])
                        nc.register_instruction(nop, overwrite=True)
                        out.append(nop)
                    inst.sync_info = _br.SyncInfo(
                        on_wait=waits[-cap:],
                        on_update=list(si.on_update) if si.on_update else [])
                out.append(inst)
            if changed:
                il.clear()
                il.extend(out)


def _rope_tiles(nc, dst, src, cos_ap, sin_ap, t1, t2, sin_swapped=False):
    """dst = src*cos + rot(src)*sin, in [hd, n] layout. src may be PSUM
    (then sin table is sign-baked: rows 0:63 hold -sin) or SBUF (then pass
    sin_swapped=True with a half-swapped table — DVE requires equal base
    partitions when both inputs are SBUF). dst is float32r SBUF."""
    nc.vector.tensor_mul(t1, src, cos_ap)
    if sin_swapped:
        # table rows 64:127 hold -sin, rows 0:63 hold +sin
        nc.vector.tensor_mul(t2[0:64, :], src[64:128, :], sin_ap[64:128, :])
        nc.vector.tensor_mul(t2[64:128, :], src[0:64, :], sin_ap[0:64, :])
    else:
        nc.vector.tensor_mul(t2[0:64, :], src[64:128, :], sin_ap[0:64, :])
        nc.vector.tensor_mul(t2[64:128, :], src[0:64, :], sin_ap[64:128, :])
    nc.vector.tensor_add(dst, t1, t2)


def _rope_sb(nc, dst, src, cos_ap, sin_ap, t2):
    """dst = src*cos + rot(src)*sin; src/table SBUF, swapped-sin layout
    (rows 0:63 = +sin, rows 64:127 = -sin). t1 lives in dst."""
    nc.vector.tensor_mul(dst, src, cos_ap)
    nc.vector.tensor_mul(t2[0:64, :], src[64:128, :], sin_ap[64:128, :])
    nc.vector.tensor_mul(t2[64:128, :], src[0:64, :], sin_ap[0:64, :])
    nc.vector.tensor_add(dst, dst, t2)


def _phase_a2(nc, tc, ktc_d, cosk_d, sink_d, ktc_r, vc_d, vc_s):
    """Cache K rope into resident fp32r tile + cache V load. Chunked so the
    scratch pool stays small enough to coexist with phase A's weight tiles
    (rope t1 writes straight into the resident ktc_r)."""
    # cache V: [KC, HD] dram -> [128, NKC*128] sbuf tile-major
    nc.sync.dma_start(
        out=vc_s[:].rearrange("p (t c) -> p t c", t=NKC),
        in_=vc_d[:, :].rearrange("(t p) c -> p t c", p=128),
    )
    CH = 512
    with tc.tile_pool(name="a2p", bufs=1) as a2p:
        for c0 in range(0, KC, CH):
            w = min(CH, KC - c0)
            sl = slice(c0, c0 + w)
            ktc_s = a2p.tile([128, CH], F32, tag="ktc")
            cosk_s = a2p.tile([128, CH], F32, tag="cosk")
            sink_s = a2p.tile([128, CH], F32, tag="sink")
            t2 = a2p.tile([128, CH], F32, tag="t2")
            nc.sync.dma_start(out=ktc_s[:, 0:w], in_=ktc_d[:, sl])
            nc.sync.dma_start(out=cosk_s[:, 0:w], in_=cosk_d[:, sl])
            nc.sync.dma_start(out=sink_s[:, 0:w], in_=sink_d[:, sl])
            # dst = src*cos + rot(src)*sin with half-swapped sin table
            nc.vector.tensor_mul(ktc_r[:, sl], ktc_s[:, 0:w], cosk_s[:, 0:w])
            nc.vector.tensor_mul(t2[0:64, 0:w], ktc_s[64:128, 0:w],
                                 sink_s[64:128, 0:w])
            nc.vector.tensor_mul(t2[64:128, 0:w], ktc_s[0:64, 0:w],
                                 sink_s[0:64, 0:w])
            nc.vector.tensor_add(ktc_r[:, sl], ktc_r[:, sl], t2[:, 0:w])


def _ex_window(t, off, ng=2):
    """AP over ex/acc tile [128, ng*QB] restricted to [off:QB] per head."""
    if off == 0:
        return t[:]
    return t[:].rearrange("p (g c) -> p g c", g=ng)[:, :, off:QB]


def _phase_b(nc, tc, qT, kcurT, v_s, ktc_r, vc_s, maskb_s, causal_s,
             onec_s, aoT, wot, out_d):
    """Attention: heads processed in 2 groups of 2; exp batched per group
    ([128, 2*QB] per ACT instruction, one PSUM 2-bank sc tile per kt).
    Softmax denominator: ex tiles accumulate on DVE + Pool(gpsimd) into SBUF
    acc pairs; PE folds accs with ones^T at group end (den PSUM holds only
    the [1, QB] folds). Group finalize (last AV, folds, normalize) is emitted
    AFTER the next group's first score/exp so neither PE nor ACT idles at
    group boundaries."""
    NG = 2  # heads per group
    with tc.tile_pool(name="ex", bufs=8) as expool, \
         tc.tile_pool(name="acd", bufs=2) as acd, \
         tc.tile_pool(name="acp", bufs=1) as acp, \
         tc.tile_pool(name="nrm", bufs=3) as nrm, \
         tc.tile_pool(name="nrmr", bufs=2) as nrmr, \
         tc.tile_pool(name="drs", bufs=2, space="DRAM") as drs, \
         tc.tile_pool(name="ob", bufs=4) as obuf, \
         tc.tile_pool(name="scps", bufs=3, space="PSUM") as scps, \
         tc.tile_pool(name="avps", bufs=1, space="PSUM") as avps, \
         tc.tile_pool(name="cps", bufs=1, space="PSUM") as cps, \
         tc.tile_pool(name="dnps", bufs=1, space="PSUM") as dnps:
        cblocks = []     # pending o_proj (dc, qt) blocks
        cb_n = [0]

        def emit_cblock():
            if not cblocks:
                return
            dc, qt = cblocks.pop(0)
            pc = cps.tile([128, QB], F32, tag="pc")
            for ht in range(G):
                nc.tensor.matmul(
                    pc[:], aoT[ht][:, bass.ts(qt, 128)], wot[ht][dc][:],
                    start=(ht == 0), stop=(ht == G - 1))
            ob = obuf.tile([128, QB], BF16, tag="ob")
            if cb_n[0] % 2 == 0:
                nc.scalar.copy(ob[:], pc[:])
            else:
                nc.vector.tensor_copy(ob[:], pc[:])
            cb_n[0] += 1
            nc.sync.dma_start(
                out=out_d[qt * 128:(qt + 1) * 128, dc * QB:(dc + 1) * QB],
                in_=ob[:])

        class GroupCtx:
            pass

        def open_group(qb, grp):
            g = GroupCtx()
            g.qb, g.grp = qb, grp
            g.cols = bass.ts(qb, QB)
            g.nkt = NKC + G * qb + G
            g.heads = [grp * NG + i for i in range(NG)]
            g.den = [dnps.tile([128, QB], F32, tag=f"den{i}", name=f"den{i}")
                     for i in range(NG)]
            g.po = [avps.tile([128, QB], F32, tag=f"po{i}", name=f"po{i}")
                    for i in range(NG)]
            g.accd = [acd.tile([128, NG * QB], F32R, tag=f"accd{j}",
                               name=f"accd{j}") for j in range(2)]
            g.accp = [acp.tile([128, NG * QB], F32R, tag=f"accp{j}",
                               name=f"accp{j}") for j in range(4)]
            g.accp_used = POOL_DEN > 0 and g.nkt >= 2 + 2 * POOL_DEN
            g.accp_folded = False
            g.pend = None
            return g

        def kt_params(g, kt):
            cur = kt >= NKC
            c = kt - NKC
            off = max(0, c * 128 - g.qb * QB) if cur else 0
            diag = cur and c >= g.qb * (QB // 128)
            if cur:
                lv = v_s[:, bass.ts(c, 128)]
                lk = kcurT[:, bass.ts(c, 128)]
                bias = 0.0
            else:
                lk = ktc_r[:, bass.ts(kt, 128)]
                lv = vc_s[:, bass.ts(kt, 128)]
                bias = maskb_s[:, kt:kt + 1]
            return lk, lv, bias, off, diag

        def emit_sc_exp(g, kt):
            """scores + exp (+ causal mul) for kt; queues (ex, ...) on g.
            Per-head 1-bank sc tiles so phase B + woven o_proj fit in 8
            PSUM banks; exp runs per head (ACT has slack in the PE-bound
            interleaved regime)."""
            lk, lv, bias, off, diag = kt_params(g, kt)
            ex = expool.tile([128, NG * QB], F32R, tag="ex")
            for i, h in enumerate(g.heads):
                sc = scps.tile([128, QB], F32, tag="sc")
                nc.tensor.matmul(
                    sc[:, off:QB], lk,
                    qT[h][:, g.qb * QB + off:(g.qb + 1) * QB])
                nc.scalar.activation(
                    ex[:, i * QB + off:(i + 1) * QB], sc[:, off:QB],
                    AF.Exp, bias=bias, scale=SCALE)
            if diag:
                for i in range(NG):
                    nc.vector.tensor_mul(
                        ex[:, i * QB + off:i * QB + off + 128],
                        ex[:, i * QB + off:i * QB + off + 128],
                        causal_s[:])
            return (ex, lv, off, kt)

        def emit_av_den(g, pend, start, stop):
            ex, lv, off, kt = pend
            for i in range(NG):
                nc.tensor.matmul(
                    g.po[i][:, off:QB], lv,
                    ex[:, i * QB + off:(i + 1) * QB],
                    start=start, stop=stop)
            # den accumulation (cache kts are always full-width)
            if kt < 2:
                nc.vector.tensor_copy(g.accd[kt][:], ex[:])
            elif g.accp_used and kt < 2 + 2 * POOL_DEN:
                j = (kt - 2) % 4
                if kt < 6:
                    nc.gpsimd.tensor_copy(g.accp[j][:], ex[:])
                else:
                    nc.gpsimd.tensor_add(g.accp[j][:], g.accp[j][:], ex[:])
            else:
                j = kt % 2
                nc.vector.tensor_add(
                    _ex_window(g.accd[j], off), _ex_window(g.accd[j], off),
                    _ex_window(ex, off))

        def finalize_stage1(g):
            """Last AV/den, acc folds, po copy-out, reciprocal + broadcast
            DMA launch. The aoT multiply is deferred to stage2 (one group
            later) so the DMA-bounce round trip never blocks the in-order
            DVE sequencer."""
            emit_av_den(g, g.pend, start=(g.pend[3] == 0), stop=True)
            folds = [g.accd[0], g.accd[1]]
            if g.accp_used:
                folds = folds + g.accp
            for i in range(NG):
                for fi, acc in enumerate(folds):
                    nc.tensor.matmul(
                        g.den[i][0:1, :], onec_s[:],
                        acc[:, i * QB:(i + 1) * QB],
                        start=(fi == 0),
                        stop=(fi == len(folds) - 1))
            g.po_sb, g.rb_sb = [], []
            for i, h in enumerate(g.heads):
                po_sb = nrm.tile([128, QB], F32, tag=f"posb{i}",
                                 name=f"posb{i}")
                nc.vector.tensor_copy(po_sb[:], g.po[i][:])
                rec = nrmr.tile([1, QB], F32, tag="rec")
                nc.vector.reciprocal(rec[:], g.den[i][0:1, :])
                rdr = drs.tile([1, QB], F32, tag="rdr")
                nc.sync.dma_start(out=rdr[:], in_=rec[:])
                rb_sb = nrm.tile([128, QB], F32, tag="rbsb",
                                 name=f"rbsb{i}")
                rdr_ap = rdr[:]
                bcast = bass.AP(tensor=rdr_ap.tensor, offset=rdr_ap.offset,
                                ap=[[0, 128]] + list(rdr_ap.ap[1:]))
                nc.sync.dma_start(out=rb_sb[:], in_=bcast)
                g.po_sb.append(po_sb)
                g.rb_sb.append(rb_sb)

        def finalize_stage2(g):
            for i, h in enumerate(g.heads):
                nc.vector.tensor_mul(aoT[h][:, g.cols], g.po_sb[i][:],
                                     g.rb_sb[i][:])
            if g.grp == G // NG - 1 and g.qb < NQB - 1:
                # last qb's blocks run after phase B in a wider PSUM scope
                for dc in range(D // QB):
                    for qt in range(g.qb * (QB // 128),
                                    (g.qb + 1) * (QB // 128)):
                        cblocks.append((dc, qt))

        prev1 = None  # group awaiting stage1
        prev2 = None  # group awaiting stage2
        for qb in range(NQB):
            for grp in range(G // NG):
                g = open_group(qb, grp)
                for kt in range(g.nkt):
                    new = emit_sc_exp(g, kt)
                    if kt == 0 and prev1 is not None:
                        finalize_stage1(prev1)
                        prev1, prev2 = None, prev1
                    elif kt == 12 and prev2 is not None:
                        finalize_stage2(prev2)
                        prev2 = None
                    if g.pend is not None:
                        emit_av_den(g, g.pend, start=(g.pend[3] == 0),
                                    stop=False)
                    emit_cblock()
                    g.pend = new
                prev1 = g
        finalize_stage1(prev1)
        if prev2 is not None:
            finalize_stage2(prev2)
        finalize_stage2(prev1)
        while cblocks:
            emit_cblock()


def build_nc():
    nc = bass.Bass()

    # ---- DRAM I/O (per-core shards; fp32r-declared tensors feed matmuls) ----
    hsT_d = nc.dram_tensor("hsT", [D, Q], F32R, kind="ExternalInput")
    wq_d = nc.dram_tensor("wq", [D, G * HD], F32R, kind="ExternalInput")
    wk_d = nc.dram_tensor("wk", [D, HD], F32R, kind="ExternalInput")
    wv_d = nc.dram_tensor("wv", [D, HD], F32R, kind="ExternalInput")
    wo_d = nc.dram_tensor("wo", [G * HD, D], BF16, kind="ExternalInput")
    ktc_d = nc.dram_tensor("ktc", [HD, KC], F32, kind="ExternalInput")   # cache K^T (raw)
    vc_d = nc.dram_tensor("vc", [KC, HD], F32R, kind="ExternalInput")    # cache V
    cosq_d = nc.dram_tensor("cosq", [HD, Q], F32, kind="ExternalInput")
    sinq_d = nc.dram_tensor("sinq", [HD, Q], F32, kind="ExternalInput")
    cosk_d = nc.dram_tensor("cosk", [HD, KC], F32, kind="ExternalInput")
    sink_d = nc.dram_tensor("sink", [HD, KC], F32, kind="ExternalInput")
    maskb_d = nc.dram_tensor("maskb", [128, NKC], F32, kind="ExternalInput")
    causal_d = nc.dram_tensor("causal01", [128, 128], F32, kind="ExternalInput")
    onec_d = nc.dram_tensor("onec", [128, 1], F32R, kind="ExternalInput")
    ident_d = nc.dram_tensor("ident", [128, 128], F32, kind="ExternalInput")
    out_d = nc.dram_tensor("out", [Q, D], BF16, kind="ExternalOutput")

    with tile.TileContext(nc) as tc:
        # ---------------- resident tiles (live across phases) --------------
        with tc.tile_pool(name="res", bufs=1) as res, \
             tc.tile_pool(name="small", bufs=1) as small:
            qT = [res.tile([128, Q], F32R, tag=f"qT{h}", name=f"qT{h}") for h in range(G)]
            kcurT = res.tile([128, Q], F32R, tag="kcurT")
            v_s = res.tile([128, Q], F32R, tag="v_s")       # current V, [k%128, c*128+hd]
            ktc_r = res.tile([128, KC], F32R, tag="ktc_r")  # roped cache K^T
            vc_s = res.tile([128, KC], F32R, tag="vc_s")    # cache V tiles
            maskb_s = small.tile([128, NKC], F32, tag="maskb")
            causal_s = small.tile([128, 128], F32, tag="causal")
            onec_s = small.tile([128, 1], F32R, tag="onec")
            ident_s = small.tile([128, 128], F32, tag="ident")
            nc.sync.dma_start(out=maskb_s, in_=maskb_d[:, :])
            nc.sync.dma_start(out=causal_s, in_=causal_d[:, :])
            nc.sync.dma_start(out=onec_s, in_=onec_d[:, :])
            nc.sync.dma_start(out=ident_s, in_=ident_d[:, :])

            # ---------------- phase A: projections + rope ------------------
            with tc.tile_pool(name="wqkv", bufs=1) as wpool, \
                 tc.tile_pool(name="hst", bufs=4) as hpool, \
                 tc.tile_pool(name="tabq", bufs=2) as tabq, \
                 tc.tile_pool(name="scr", bufs=2) as scr, \
                 tc.tile_pool(name="sbq", bufs=1) as sbq, \
                 tc.tile_pool(name="pjps", bufs=1, space="PSUM") as pjps, \
                 tc.tile_pool(name="ptps", bufs=2, space="PSUM") as ptps:
                # weight tiles in staged bundles: small leading bundles so
                # the first projection matmul starts ~4us in, larger ones
                # after so the SP sequencer isn't the DMA bottleneck.
                WSZ = [2, 2, 4, 8, 8, 8]     # dt per weight-DMA bundle
                WSTART = [sum(WSZ[:i]) for i in range(len(WSZ))]
                WIDX = []                     # dt -> (bundle, offset)
                for bi, sz in enumerate(WSZ):
                    for o in range(sz):
                        WIDX.append((bi, o))
                wq_t = [wpool.tile([128, sz * G * HD], F32R, tag=f"wq{i}",
                                   name=f"wq{i}") for i, sz in enumerate(WSZ)]
                wk_t = [wpool.tile([128, sz * HD], F32R, tag=f"wk{i}",
                                   name=f"wk{i}") for i, sz in enumerate(WSZ)]
                wv_t = [wpool.tile([128, sz * HD], F32R, tag=f"wv{i}",
                                   name=f"wv{i}") for i, sz in enumerate(WSZ)]

                for qb in range(NQB):
                    cols = bass.ts(qb, QB)
                    cosq_s = tabq.tile([128, QB], F32, tag="cosq")
                    sinq_s = tabq.tile([128, QB], F32, tag="sinq")
                    pq = [pjps.tile([128, QB], F32, tag=f"pq{h}", name=f"pq{h}") for h in range(G)]
                    pk = pjps.tile([128, QB], F32, tag="pk")
                    pv = pjps.tile([128, QB], F32, tag="pv")
                    for dt in range(NDT):
                        rows = bass.ts(dt, 128)
                        wi, wo_ = WIDX[dt]
                        if qb == 0 and wo_ == 0:
                            sz = WSZ[wi]
                            wrows = slice(WSTART[wi] * 128,
                                          (WSTART[wi] + sz) * 128)
                            nc.sync.dma_start(
                                out=wk_t[wi][:].rearrange(
                                    "p (t c) -> p t c", t=sz),
                                in_=wk_d[wrows, :].rearrange(
                                    "(t p) c -> p t c", p=128))
                            nc.sync.dma_start(
                                out=wv_t[wi][:].rearrange(
                                    "p (t c) -> p t c", t=sz),
                                in_=wv_d[wrows, :].rearrange(
                                    "(t p) c -> p t c", p=128))
                            nc.sync.dma_start(
                                out=wq_t[wi][:].rearrange(
                                    "p (t c) -> p t c", t=sz),
                                in_=wq_d[wrows, :].rearrange(
                                    "(t p) c -> p t c", p=128))
                        hst = hpool.tile([128, QB], F32R, tag="hst")
                        nc.sync.dma_start(out=hst, in_=hsT_d[rows, cols])
                        if dt == 0:
                            nc.sync.dma_start(out=cosq_s, in_=cosq_d[:, cols])
                            nc.sync.dma_start(out=sinq_s, in_=sinq_d[:, cols])
                        st = dict(start=(dt == 0), stop=(dt == NDT - 1))
                        for h in range(G):
                            nc.tensor.matmul(
                                pq[h][:],
                                wq_t[wi][:, wo_ * G * HD + h * HD:
                                          wo_ * G * HD + (h + 1) * HD],
                                hst[:], **st)
                        nc.tensor.matmul(
                            pk[:], wk_t[wi][:, bass.ts(wo_, HD)], hst[:], **st)
                        nc.tensor.matmul(
                            pv[:], wv_t[wi][:, bass.ts(wo_, HD)], hst[:], **st)
                    # drain projection PSUM fast via ACT copies (frees the
                    # banks for the next qb ~0.6us after the last matmul),
                    # then rope lazily on DVE from the SBUF copies using the
                    # half-swapped sin table.
                    qsb = []
                    for h in range(G):
                        t = sbq.tile([128, QB], F32, tag=f"qsb{h}",
                                     name=f"qsb{h}")
                        nc.scalar.copy(t[:], pq[h][:])
                        qsb.append(t)
                    ksb = sbq.tile([128, QB], F32, tag="ksb")
                    nc.scalar.copy(ksb[:], pk[:])
                    vT_sb = scr.tile([128, QB], F32, tag="vT")
                    nc.scalar.copy(vT_sb[:], pv[:])
                    for j in range(QB // 128):
                        pst = ptps.tile([128, 128], F32, tag="pst")
                        nc.tensor.transpose(
                            pst[:], vT_sb[:, bass.ts(j, 128)], ident_s[:])
                        c = qb * (QB // 128) + j
                        nc.scalar.copy(v_s[:, bass.ts(c, 128)], pst[:])
                    for h in range(G):
                        t2 = scr.tile([128, QB], F32, tag="t2")
                        _rope_sb(nc, qT[h][:, cols], qsb[h][:], cosq_s[:],
                                 sinq_s[:], t2[:])
                    t2 = scr.tile([128, QB], F32, tag="t2")
                    _rope_sb(nc, kcurT[:, cols], ksb[:], cosq_s[:],
                             sinq_s[:], t2[:])
                    if qb == 0:
                        # cache K/V load + rope: DMAs queue behind qb0's
                        # weights, DVE rope overlaps qb1's projections.
                        _phase_a2(nc, tc, ktc_d, cosk_d, sink_d, ktc_r,
                                  vc_d, vc_s)

            # ---------------- phase B: attention ---------------------------
            # aoT allocated here (not in res) so phase A can keep Wq resident.
            with tc.tile_pool(name="aob", bufs=1) as aob, \
                 tc.tile_pool(name="wost", bufs=1) as wopool:
                aoT = [aob.tile([128, Q], BF16, tag=f"aoT{h}", name=f"aoT{h}")
                       for h in range(G)]
                # all o_proj weights resident in bf16, loaded during phase B
                # (DMA is idle there); o_proj for qb<3 is woven into phase
                # B's kt loop to fill the PE bubbles of the exp-bound phase.
                wot = [[wopool.tile([128, QB], BF16, tag=f"wot{ht}_{dc}",
                                    name=f"wot{ht}_{dc}")
                        for dc in range(D // QB)] for ht in range(G)]
                for ht in range(G):
                    for dc in range(D // QB):
                        nc.sync.dma_start(
                            out=wot[ht][dc],
                            in_=wo_d[ht * 128:(ht + 1) * 128,
                                     dc * QB:(dc + 1) * QB])
                _phase_b(nc, tc, qT, kcurT, v_s, ktc_r, vc_s, maskb_s,
                         causal_s, onec_s, aoT, wot, out_d)
                # ---- tail o_proj: the last qb's blocks, 4-bank pipelined ---
                with tc.tile_pool(name="obt", bufs=4) as obuf2, \
                     tc.tile_pool(name="cpst", bufs=4, space="PSUM") as cps2:
                    for dc in range(D // QB):
                        for qt in range((NQB - 1) * (QB // 128), Q // 128):
                            pc = cps2.tile([128, QB], F32, tag="pc")
                            for ht in range(G):
                                nc.tensor.matmul(
                                    pc[:], aoT[ht][:, bass.ts(qt, 128)],
                                    wot[ht][dc][:],
                                    start=(ht == 0), stop=(ht == G - 1))
                            ob = obuf2.tile([128, QB], BF16, tag="ob")
                            if (dc + qt) % 2 == 0:
                                nc.scalar.copy(ob[:], pc[:])
                            else:
                                nc.vector.tensor_copy(ob[:], pc[:])
                            nc.sync.dma_start(
                                out=out_d[qt * 128:(qt + 1) * 128,
                                          dc * QB:(dc + 1) * QB],
                                in_=ob[:])
    _split_excess_waits(nc)
    return nc


_NC_CACHE = None


def _get_nc():
    global _NC_CACHE
    if _NC_CACHE is None:
        _NC_CACHE = build_nc()
    return _NC_CACHE


def _tables(pos):
    """cos/sin tables in [hd, n] layout; sin rows 0:63 negated (rope rot)."""
    inv_freq = 1.0 / (ROPE_BASE ** (np.arange(0, HD, 2, dtype=np.float32)
                                    / np.float32(HD)))
    inv_freq = inv_freq.astype(np.float32)
    ang = (pos.astype(np.float32)[None, :] * inv_freq[:, None]).astype(np.float32)
    a64 = ang.astype(np.float64)
    cos = np.cos(a64).astype(np.float32)
    sin = np.sin(a64).astype(np.float32)
    cosT = np.concatenate([cos, cos], axis=0)
    sinT = np.concatenate([-sin, sin], axis=0)
    return np.ascontiguousarray(cosT), np.ascontiguousarray(sinT)


def _prepare_in_maps(hidden_states, sink_k, sink_v, win_k, win_v, sink_pos,
                     key_pos, sink_mask, key_mask, Wq, Wk, Wv, Wo):
    hs = np.asarray(hidden_states, dtype=np.float32)[0]        # [Q, D]
    hsT = np.ascontiguousarray(hs.T)                            # [D, Q]
    Wq = np.asarray(Wq, dtype=np.float32)
    Wk = np.asarray(Wk, dtype=np.float32)
    Wv = np.asarray(Wv, dtype=np.float32)
    Wo = np.asarray(Wo, dtype=np.float32)
    sink_k = np.asarray(sink_k, dtype=np.float32)
    sink_v = np.asarray(sink_v, dtype=np.float32)
    win_k = np.asarray(win_k, dtype=np.float32)
    win_v = np.asarray(win_v, dtype=np.float32)
    spos = np.asarray(sink_pos).astype(np.int64)
    kpos = np.asarray(key_pos).astype(np.int64)
    smask = np.asarray(sink_mask, dtype=np.float32)
    kmask = np.asarray(key_mask, dtype=np.float32)

    max_pos = max(int(spos.max()), int(kpos.max())) + 1
    qpos = np.arange(Q, dtype=np.float64) + max_pos
    cosq, sinq = _tables(qpos)                                  # [128, Q]
    # q/k ropes read SBUF->SBUF: swap sin halves (rows 0:63 = +sin,
    # rows 64:127 = -sin) so DVE base partitions align
    sinq = np.ascontiguousarray(
        np.concatenate([-sinq[0:64], -sinq[64:128]], axis=0))
    cache_pos = np.concatenate([spos.astype(np.float64),
                                kpos.astype(np.float64),
                                np.zeros(KC - NS - NW)])
    cosk, sink_t = _tables(cache_pos)                           # [128, KC]
    # cache rope reads SBUF->SBUF: swap sin halves so base partitions align
    sink_t = np.ascontiguousarray(
        np.concatenate([-sink_t[0:64], -sink_t[64:128]], axis=0))

    maskb = np.concatenate([smask, kmask,
                            np.ones(KC - NS - NW, np.float32)]).astype(np.float32)
    maskb = maskb * np.float32(NEG)
    maskb_T = np.ascontiguousarray(maskb.reshape(NKC, 128).T)   # [128, NKC]

    causal01 = (np.arange(128)[:, None] <= np.arange(128)[None, :]) \
        .astype(np.float32)                                     # keep k<=q
    onec = np.ones((128, 1), np.float32)
    ident = np.eye(128, dtype=np.float32)

    Wq_h = Wq.reshape(D, H, HD)
    Wo_h = Wo.reshape(H, HD, D)
    pad = KC - NS - NW

    in_maps = []
    for c in range(NC_CORES):
        hsel = slice(c * G, (c + 1) * G)
        wq_c = np.ascontiguousarray(Wq_h[:, hsel].reshape(D, G * HD))
        wk_c = np.ascontiguousarray(Wk[:, c * HD:(c + 1) * HD])
        wv_c = np.ascontiguousarray(Wv[:, c * HD:(c + 1) * HD])
        wo_c = np.ascontiguousarray(
            Wo_h[hsel].reshape(G * HD, D).astype(ml_dtypes.bfloat16))
        kc = np.concatenate([sink_k[0, c], win_k[0, c],
                             np.zeros((pad, HD), np.float32)], axis=0)  # [KC, HD]
        ktc = np.ascontiguousarray(kc.T)                                # [HD, KC]
        vc = np.concatenate([sink_v[0, c], win_v[0, c],
                             np.zeros((pad, HD), np.float32)], axis=0)
        in_maps.append(dict(
            hsT=hsT, wq=wq_c, wk=wk_c, wv=wv_c, wo=wo_c,
            ktc=ktc, vc=np.ascontiguousarray(vc),
            cosq=cosq, sinq=sinq, cosk=cosk, sink=sink_t,
            maskb=maskb_T, causal01=causal01,
            onec=onec, ident=ident,
        ))

    return in_maps


def kernel(**inputs):
    in_maps = _prepare_in_maps(**inputs)
    nc = _get_nc()
    res = run_bass_kernel_spmd(nc, in_maps, list(range(NC_CORES)))
    acc = np.zeros((Q, D), dtype=np.float64)
    for r in res.results:
        acc += np.asarray(r["out"], dtype=np.float64)
    return acc.astype(np.float32)[None]


if __name__ == "__main__":
    nc = build_nc()
    ni = sum(len(bb.instructions) for f in nc.m.functions for bb in f.blocks)
    print(f"built ok: {ni} instructions")


# revision 36
# speedup vs baseline: 1.0140x; 1.0140x over previous
"""CascadeAttention TRN2 kernel — 8-core head-sharded tensor parallel.

Sharding: each of the 8 NeuronCores owns 4 query heads + 1 KV head (GQA group).
Per core: qkv projections (fp32r matmuls), RoPE, cascade attention over
(sink + window + current) keys with causal masking on the current block,
softmax without max-subtraction (scores are small), o_proj partial product.
Host: shards weights/caches per head, precomputes cos/sin position tables
(ACT Sin has no range reduction on large angles), transposes hidden_states
once (activations must enter the PE contraction-dim-major), and sums the 8
o_proj partials.

All matmuls run in float32r (TF32-like, 1 cycle/row at N>=256 vs 4 for fp32;
measured relerr 1.6e-4) — inputs are either DMA'd into float32r-declared
tensors or written as float32r by the producing compute op, which is what the
walrus FP32r verifier requires.

Schedule notes (v2):
- Weight DMAs are per-dt tiles interleaved with qb0's hst stream so the first
  projection matmul starts ~2us in instead of waiting 40us for bulk weights.
- Cache K/V load + rope (A2) is emitted mid-phase-A so it overlaps the
  projection matmuls instead of serializing between phases A and B.
- Softmax denominator accumulation is off the PE: cache-tile ex sums run as
  DVE/Pool(gpsimd) tensor_add chains into SBUF accumulators; the PE only
  folds the accumulators (ones^T @ acc) at group end. This turns phase B
  from PE-bound into ACT(exp)-bound.
- o_proj partials are written as bf16 (host upconverts and sums), halving
  phase-C write traffic.
"""
import os
import sys

for _p in ("/root/.axon_site/_ro/trn_rl_repo", "/opt/trn_rl_repo"):
    if os.path.isdir(_p) and _p not in sys.path:
        sys.path.insert(0, _p)

import ml_dtypes
import numpy as np

import concourse.bass as bass
import concourse.mybir as mybir
import concourse.tile as tile
from concourse.bass_utils import run_bass_kernel_spmd
from concourse.vector_clock import ScopedClock, VectorClock

F32 = mybir.dt.float32
F32R = mybir.dt.float32r
BF16 = mybir.dt.bfloat16
AF = mybir.ActivationFunctionType

B, Q, D = 1, 2048, 4096
H, KVH, HD = 32, 8, 128
NS, NW = 4, 2048
G = H // KVH           # q heads per kv head = heads per core
NC_CORES = 8
ROPE_BASE = 10000.0

QB = 512               # q block (matmul moving dim)
NQB = Q // QB          # 4
NDT = D // 128         # 32 contraction tiles
NKC = 17               # cache key tiles: 4 sink + 2048 window + 124 pad = 2176
KC = NKC * 128         # 2176
SCALE = 1.0 / float(np.sqrt(HD))
NEG = float(np.finfo(np.float32).min)

# den accumulation split: cache kt in [0,2) init the DVE acc pair; cache kt in
# [2, 2+2*POOL_DEN) go to the Pool(gpsimd) acc pair; the rest of the cache kts
# and all current kts run on the DVE pair.
POOL_DEN = 4           # cache kts per acc on the Pool pair (2 accs -> 8 kts)


# ---------------------------------------------------------------------------
# TileContext tail-drain patch: stock _drain_and_barrier puts one sync-wait per
# outstanding processor on a single SP Drain, overflowing walrus's per-
# instruction wait slots. Split the waits across per-proc SP NoOps instead.
def _split_drain_and_barrier(self, tick_clock, wait_clock):
    nc = self.nc
    gc = tick_clock.global_clock
    n = len(gc)
    for i in range(n):
        t = gc[i]
        if t > 0:
            vec = [0] * n
            vec[i] = t
            nop = nc.sync.nop(nofuse=True, hint=f"tail_wait_p{i}")
            wait_clock.add_sem_waits(nop.ins, ScopedClock({None: VectorClock(vec)}))
    drain_inst = nc.sync.drain()
    full = ScopedClock({None: tick_clock.global_clock})
    wait_clock.add_sem_waits(drain_inst.ins, full, full.copy())
    nc.all_engine_barrier()
    assert self.sems is not None
    popped = nc._tile_sem_poison_stack.pop()
    assert popped is self._sem_poison
    nc.clear_and_free_semaphores(list(self.sems.allocated().values()))
    nc.all_engine_barrier()


tile.TileContext._drain_and_barrier = _split_drain_and_barrier


def _split_excess_waits(nc, cap=1):
    """Walrus enforces small per-instruction sync-wait limits (1-2 depending
    on the lowered encoding). Tile emits up to ~4 on body instructions and
    more on drains. Move excess waits onto same-engine NoOps placed directly
    before the instruction — sems are monotonic in the kernel body, so
    waiting earlier on the same engine is semantically identical."""
    import bass_rust as _br
    for f in nc.m.functions:
        for bb in f.blocks:
            il = bb.instructions
            out = []
            changed = False
            for inst in il:
                si = inst.sync_info
                waits = list(si.on_wait) if (si is not None and si.on_wait) else []
                if len(waits) > cap:
                    changed = True
                    for j, w in enumerate(waits[:-cap]):
                        nop = mybir.InstNoOp(
                            name=f"{inst.name}-w{j}", ins=[], outs=[])
                        nop.engine = inst.engine
                        nop.sync_info = _br.SyncInfo(on_wait=[w], on_update=[])
                        nc.register_instruction(nop, overwrite=True)
                        out.append(nop)
                    inst.sync_info = _br.SyncInfo(
                        on_wait=waits[-cap:],
                        on_update=list(si.on_update) if si.on_update else [])
                out.append(inst)
            if changed:
                il.clear()
                il.extend(out)


def _rope_tiles(nc, dst, src, cos_ap, sin_ap, t1, t2, sin_swapped=False):
    """dst = src*cos + rot(src)*sin, in [hd, n] layout. src may be PSUM
    (then sin table is sign-baked: rows 0:63 hold -sin) or SBUF (then pass
    sin_swapped=True with a half-swapped table — DVE requires equal base
    partitions when both inputs are SBUF). dst is float32r SBUF."""
    nc.vector.tensor_mul(t1, src, cos_ap)
    if sin_swapped:
        # table rows 64:127 hold -sin, rows 0:63 hold +sin
        nc.vector.tensor_mul(t2[0:64, :], src[64:128, :], sin_ap[64:128, :])
        nc.vector.tensor_mul(t2[64:128, :], src[0:64, :], sin_ap[0:64, :])
    else:
        nc.vector.tensor_mul(t2[0:64, :], src[64:128, :], sin_ap[0:64, :])
        nc.vector.tensor_mul(t2[64:128, :], src[0:64, :], sin_ap[64:128, :])
    nc.vector.tensor_add(dst, t1, t2)


def _rope_sb(nc, dst, src, cos_ap, sin_ap, t2):
    """dst = src*cos + rot(src)*sin; src/table SBUF, swapped-sin layout
    (rows 0:63 = +sin, rows 64:127 = -sin). t1 lives in dst."""
    nc.vector.tensor_mul(dst, src, cos_ap)
    nc.vector.tensor_mul(t2[0:64, :], src[64:128, :], sin_ap[64:128, :])
    nc.vector.tensor_mul(t2[64:128, :], src[0:64, :], sin_ap[0:64, :])
    nc.vector.tensor_add(dst, dst, t2)


def _phase_a2(nc, tc, ktc_d, cosk_d, sink_d, ktc_r, vc_d, vc_s, a2p,
              half):
    """Cache K rope into resident fp32r tile + cache V load, emitted in two
    halves (after qb1 and qb2) so its DMA queue load spreads across two
    blocks' DMA slack. Chunked so the scratch pool stays small enough to
    coexist with phase A's weight tiles."""
    CH = 256
    if half == 0:
        # cache V: [KC, HD] dram -> [128, NKC*128] sbuf tile-major
        nc.sync.dma_start(
            out=vc_s[:].rearrange("p (t c) -> p t c", t=NKC),
            in_=vc_d[:, :].rearrange("(t p) c -> p t c", p=128),
        )
    if True:
        nch = (KC + CH - 1) // CH
        lo = 0 if half == 0 else (nch // 2) * CH
        hi = (nch // 2) * CH if half == 0 else KC
        for c0 in range(lo, hi, CH):
            w = min(CH, KC - c0)
            sl = slice(c0, c0 + w)
            ktc_s = a2p.tile([128, CH], F32, tag="ktc")
            cosk_s = a2p.tile([128, CH], F32, tag="cosk")
            sink_s = a2p.tile([128, CH], F32, tag="sink")
            t2 = a2p.tile([128, CH], F32, tag="t2")
            nc.sync.dma_start(out=ktc_s[:, 0:w], in_=ktc_d[:, sl])
            nc.sync.dma_start(out=cosk_s[:, 0:w], in_=cosk_d[:, sl])
            nc.sync.dma_start(out=sink_s[:, 0:w], in_=sink_d[:, sl])
            # dst = src*cos + rot(src)*sin with half-swapped sin table
            nc.vector.tensor_mul(ktc_r[:, sl], ktc_s[:, 0:w], cosk_s[:, 0:w])
            nc.vector.tensor_mul(t2[0:64, 0:w], ktc_s[64:128, 0:w],
                                 sink_s[64:128, 0:w])
            nc.vector.tensor_mul(t2[64:128, 0:w], ktc_s[0:64, 0:w],
                                 sink_s[0:64, 0:w])
            nc.vector.tensor_add(ktc_r[:, sl], ktc_r[:, sl], t2[:, 0:w])


def _ex_window(t, off, ng=2):
    """AP over ex/acc tile [128, ng*QB] restricted to [off:QB] per head."""
    if off == 0:
        return t[:]
    return t[:].rearrange("p (g c) -> p g c", g=ng)[:, :, off:QB]


def _phase_b(nc, tc, qT, kcurT, v_s, ktc_r, vc_s, maskb_s, causal_s,
             onec_s, aoT, wot, out_d):
    """Attention: heads processed in 2 groups of 2; exp batched per group
    ([128, 2*QB] per ACT instruction, one PSUM 2-bank sc tile per kt).
    Softmax denominator: ex tiles accumulate on DVE + Pool(gpsimd) into SBUF
    acc pairs; PE folds accs with ones^T at group end (den PSUM holds only
    the [1, QB] folds). Group finalize (last AV, folds, normalize) is emitted
    AFTER the next group's first score/exp so neither PE nor ACT idles at
    group boundaries."""
    NG = 2  # heads per group
    with tc.tile_pool(name="ex", bufs=8) as expool, \
         tc.tile_pool(name="acd", bufs=2) as acd, \
         tc.tile_pool(name="acp", bufs=1) as acp, \
         tc.tile_pool(name="nrm", bufs=3) as nrm, \
         tc.tile_pool(name="nrmr", bufs=2) as nrmr, \
         tc.tile_pool(name="drs", bufs=2, space="DRAM") as drs, \
         tc.tile_pool(name="ob", bufs=4) as obuf, \
         tc.tile_pool(name="scps", bufs=3, space="PSUM") as scps, \
         tc.tile_pool(name="avps", bufs=1, space="PSUM") as avps, \
         tc.tile_pool(name="cps", bufs=1, space="PSUM") as cps, \
         tc.tile_pool(name="dnps", bufs=1, space="PSUM") as dnps:
        cblocks = []     # pending o_proj (dc, qt) blocks
        cb_n = [0]

        def emit_cblock():
            if not cblocks:
                return
            dc, qt = cblocks.pop(0)
            pc = cps.tile([128, QB], F32, tag="pc")
            for ht in range(G):
                nc.tensor.matmul(
                    pc[:], aoT[ht][:, bass.ts(qt, 128)], wot[ht][dc][:],
                    start=(ht == 0), stop=(ht == G - 1))
            ob = obuf.tile([128, QB], BF16, tag="ob")
            if cb_n[0] % 2 == 0:
                nc.scalar.copy(ob[:], pc[:])
            else:
                nc.vector.tensor_copy(ob[:], pc[:])
            cb_n[0] += 1
            nc.sync.dma_start(
                out=out_d[qt * 128:(qt + 1) * 128, dc * QB:(dc + 1) * QB],
                in_=ob[:])

        class GroupCtx:
            pass

        def open_group(qb, grp):
            g = GroupCtx()
            g.qb, g.grp = qb, grp
            g.cols = bass.ts(qb, QB)
            g.nkt = NKC + G * qb + G
            g.heads = [grp * NG + i for i in range(NG)]
            g.den = [dnps.tile([128, QB], F32, tag=f"den{i}", name=f"den{i}")
                     for i in range(NG)]
            g.po = [avps.tile([128, QB], F32, tag=f"po{i}", name=f"po{i}")
                    for i in range(NG)]
            g.accd = [acd.tile([128, NG * QB], F32R, tag=f"accd{j}",
                               name=f"accd{j}") for j in range(2)]
            g.accp = [acp.tile([128, NG * QB], F32R, tag=f"accp{j}",
                               name=f"accp{j}") for j in range(4)]
            g.accp_used = POOL_DEN > 0 and g.nkt >= 2 + 2 * POOL_DEN
            g.accp_folded = False
            g.pend = None
            return g

        def kt_params(g, kt):
            cur = kt >= NKC
            c = kt - NKC
            off = max(0, c * 128 - g.qb * QB) if cur else 0
            diag = cur and c >= g.qb * (QB // 128)
            if cur:
                lv = v_s[:, bass.ts(c, 128)]
                lk = kcurT[:, bass.ts(c, 128)]
                bias = 0.0
            else:
                lk = ktc_r[:, bass.ts(kt, 128)]
                lv = vc_s[:, bass.ts(kt, 128)]
                bias = maskb_s[:, kt:kt + 1]
            return lk, lv, bias, off, diag

        def emit_sc_exp(g, kt):
            """scores + exp (+ causal mul) for kt; queues (ex, ...) on g.
            Per-head 1-bank sc tiles so phase B + woven o_proj fit in 8
            PSUM banks; exp runs per head (ACT has slack in the PE-bound
            interleaved regime)."""
            lk, lv, bias, off, diag = kt_params(g, kt)
            ex = expool.tile([128, NG * QB], F32R, tag="ex")
            for i, h in enumerate(g.heads):
                sc = scps.tile([128, QB], F32, tag="sc")
                nc.tensor.matmul(
                    sc[:, off:QB], lk,
                    qT[h][:, g.qb * QB + off:(g.qb + 1) * QB])
                nc.scalar.activation(
                    ex[:, i * QB + off:(i + 1) * QB], sc[:, off:QB],
                    AF.Exp, bias=bias, scale=SCALE)
            if diag:
                for i in range(NG):
                    nc.vector.tensor_mul(
                        ex[:, i * QB + off:i * QB + off + 128],
                        ex[:, i * QB + off:i * QB + off + 128],
                        causal_s[:])
            return (ex, lv, off, kt)

        def emit_av_den(g, pend, start, stop):
            ex, lv, off, kt = pend
            for i in range(NG):
                nc.tensor.matmul(
                    g.po[i][:, off:QB], lv,
                    ex[:, i * QB + off:(i + 1) * QB],
                    start=start, stop=stop)
            # den accumulation (cache kts are always full-width)
            if kt < 2:
                nc.vector.tensor_copy(g.accd[kt][:], ex[:])
            elif g.accp_used and kt < 2 + 2 * POOL_DEN:
                j = (kt - 2) % 4
                if kt < 6:
                    nc.gpsimd.tensor_copy(g.accp[j][:], ex[:])
                else:
                    nc.gpsimd.tensor_add(g.accp[j][:], g.accp[j][:], ex[:])
            else:
                j = kt % 2
                nc.vector.tensor_add(
                    _ex_window(g.accd[j], off), _ex_window(g.accd[j], off),
                    _ex_window(ex, off))

        def finalize_stage1(g):
            """Last AV/den, acc folds, po copy-out, reciprocal + broadcast
            DMA launch. The aoT multiply is deferred to stage2 (one group
            later) so the DMA-bounce round trip never blocks the in-order
            DVE sequencer."""
            emit_av_den(g, g.pend, start=(g.pend[3] == 0), stop=True)
            folds = [g.accd[0], g.accd[1]]
            if g.accp_used:
                folds = folds + g.accp
            for i in range(NG):
                for fi, acc in enumerate(folds):
                    nc.tensor.matmul(
                        g.den[i][0:1, :], onec_s[:],
                        acc[:, i * QB:(i + 1) * QB],
                        start=(fi == 0),
                        stop=(fi == len(folds) - 1))
            g.po_sb, g.rb_sb = [], []
            for i, h in enumerate(g.heads):
                po_sb = nrm.tile([128, QB], F32, tag=f"posb{i}",
                                 name=f"posb{i}")
                nc.vector.tensor_copy(po_sb[:], g.po[i][:])
                rec = nrmr.tile([1, QB], F32, tag="rec")
                nc.vector.reciprocal(rec[:], g.den[i][0:1, :])
                rdr = drs.tile([1, QB], F32, tag="rdr")
                nc.sync.dma_start(out=rdr[:], in_=rec[:])
                rb_sb = nrm.tile([128, QB], F32, tag="rbsb",
                                 name=f"rbsb{i}")
                rdr_ap = rdr[:]
                bcast = bass.AP(tensor=rdr_ap.tensor, offset=rdr_ap.offset,
                                ap=[[0, 128]] + list(rdr_ap.ap[1:]))
                nc.sync.dma_start(out=rb_sb[:], in_=bcast)
                g.po_sb.append(po_sb)
                g.rb_sb.append(rb_sb)

        def finalize_stage2(g):
            for i, h in enumerate(g.heads):
                nc.vector.tensor_mul(aoT[h][:, g.cols], g.po_sb[i][:],
                                     g.rb_sb[i][:])
            if g.grp == G // NG - 1 and g.qb < NQB - 1:
                # last qb's blocks run after phase B in a wider PSUM scope
                for dc in range(D // QB):
                    for qt in range(g.qb * (QB // 128),
                                    (g.qb + 1) * (QB // 128)):
                        cblocks.append((dc, qt))

        prev1 = None  # group awaiting stage1
        prev2 = None  # group awaiting stage2
        for qb in range(NQB):
            for grp in range(G // NG):
                g = open_group(qb, grp)
                for kt in range(g.nkt):
                    new = emit_sc_exp(g, kt)
                    if kt == 0 and prev1 is not None:
                        finalize_stage1(prev1)
                        prev1, prev2 = None, prev1
                    elif kt == 12 and prev2 is not None:
                        finalize_stage2(prev2)
                        prev2 = None
                    if g.pend is not None:
                        emit_av_den(g, g.pend, start=(g.pend[3] == 0),
                                    stop=False)
                    emit_cblock()
                    g.pend = new
                prev1 = g
        finalize_stage1(prev1)
        if prev2 is not None:
            finalize_stage2(prev2)
        finalize_stage2(prev1)
        while cblocks:
            emit_cblock()


def build_nc():
    nc = bass.Bass()

    # ---- DRAM I/O (per-core shards; fp32r-declared tensors feed matmuls) ----
    hsT_d = nc.dram_tensor("hsT", [D, Q], F32R, kind="ExternalInput")
    wq_d = nc.dram_tensor("wq", [D, G * HD], BF16, kind="ExternalInput")
    wk_d = nc.dram_tensor("wk", [D, HD], BF16, kind="ExternalInput")
    wv_d = nc.dram_tensor("wv", [D, HD], BF16, kind="ExternalInput")
    wo_d = nc.dram_tensor("wo", [G * HD, D], BF16, kind="ExternalInput")
    ktc_d = nc.dram_tensor("ktc", [HD, KC], F32, kind="ExternalInput")   # cache K^T (raw)
    vc_d = nc.dram_tensor("vc", [KC, HD], F32R, kind="ExternalInput")    # cache V
    cosq_d = nc.dram_tensor("cosq", [HD, Q], F32, kind="ExternalInput")
    sinq_d = nc.dram_tensor("sinq", [HD, Q], F32, kind="ExternalInput")
    cosk_d = nc.dram_tensor("cosk", [HD, KC], F32, kind="ExternalInput")
    sink_d = nc.dram_tensor("sink", [HD, KC], F32, kind="ExternalInput")
    maskb_d = nc.dram_tensor("maskb", [128, NKC], F32, kind="ExternalInput")
    causal_d = nc.dram_tensor("causal01", [128, 128], F32, kind="ExternalInput")
    onec_d = nc.dram_tensor("onec", [128, 1], F32R, kind="ExternalInput")
    ident_d = nc.dram_tensor("ident", [128, 128], F32, kind="ExternalInput")
    out_d = nc.dram_tensor("out", [Q, D], BF16, kind="ExternalOutput")

    with tile.TileContext(nc) as tc:
        # ---------------- resident tiles (live across phases) --------------
        with tc.tile_pool(name="res", bufs=1) as res, \
             tc.tile_pool(name="small", bufs=1) as small:
            qT = [res.tile([128, Q], F32R, tag=f"qT{h}", name=f"qT{h}") for h in range(G)]
            kcurT = res.tile([128, Q], F32R, tag="kcurT")
            v_s = res.tile([128, Q], F32R, tag="v_s")       # current V, [k%128, c*128+hd]
            ktc_r = res.tile([128, KC], F32R, tag="ktc_r")  # roped cache K^T
            vc_s = res.tile([128, KC], F32R, tag="vc_s")    # cache V tiles
            maskb_s = small.tile([128, NKC], F32, tag="maskb")
            causal_s = small.tile([128, 128], F32, tag="causal")
            onec_s = small.tile([128, 1], F32R, tag="onec")
            ident_s = small.tile([128, 128], F32, tag="ident")
            nc.sync.dma_start(out=maskb_s, in_=maskb_d[:, :])
            nc.sync.dma_start(out=causal_s, in_=causal_d[:, :])
            nc.sync.dma_start(out=onec_s, in_=onec_d[:, :])
            nc.sync.dma_start(out=ident_s, in_=ident_d[:, :])

            # ---------------- phase A: projections + rope ------------------
            with tc.tile_pool(name="wqkv", bufs=1) as wpool, \
                 tc.tile_pool(name="hst", bufs=4) as hpool, \
                 tc.tile_pool(name="tabq", bufs=2) as tabq, \
                 tc.tile_pool(name="scr", bufs=2) as scr, \
                 tc.tile_pool(name="sbq", bufs=1) as sbq, \
                 tc.tile_pool(name="wstg", bufs=1) as wstg, \
                 tc.tile_pool(name="a2p", bufs=1) as a2p, \
                 tc.tile_pool(name="pjps", bufs=1, space="PSUM") as pjps, \
                 tc.tile_pool(name="ptps", bufs=2, space="PSUM") as ptps:
                # weight tiles in staged bundles: small leading bundles so
                # the first projection matmul starts ~4us in, larger ones
                # after so the SP sequencer isn't the DMA bottleneck.
                WSZ = [2, 2, 4, 8, 8, 8]     # dt per weight-DMA bundle
                WSTART = [sum(WSZ[:i]) for i in range(len(WSZ))]
                WIDX = []                     # dt -> (bundle, offset)
                for bi, sz in enumerate(WSZ):
                    for o in range(sz):
                        WIDX.append((bi, o))
                wq_t = [wpool.tile([128, sz * G * HD], F32R, tag=f"wq{i}",
                                   name=f"wq{i}") for i, sz in enumerate(WSZ)]
                wk_t = [wpool.tile([128, sz * HD], F32R, tag=f"wk{i}",
                                   name=f"wk{i}") for i, sz in enumerate(WSZ)]
                wv_t = [wpool.tile([128, sz * HD], F32R, tag=f"wv{i}",
                                   name=f"wv{i}") for i, sz in enumerate(WSZ)]

                for qb in range(NQB):
                    cols = bass.ts(qb, QB)
                    cosq_s = tabq.tile([128, QB], F32, tag="cosq")
                    sinq_s = tabq.tile([128, QB], F32, tag="sinq")
                    pq = [pjps.tile([128, QB], F32, tag=f"pq{h}", name=f"pq{h}") for h in range(G)]
                    pk = pjps.tile([128, QB], F32, tag="pk")
                    pv = pjps.tile([128, QB], F32, tag="pv")
                    for dt in range(NDT):
                        rows = bass.ts(dt, 128)
                        wi, wo_ = WIDX[dt]
                        if qb == 0 and wo_ == 0:
                            # weights arrive bf16 (halves qb0's DMA-bound
                            # prefix) and are upconverted on the idle DVE
                            # into the f32r tiles the matmuls read; wq in
                            # half-bundles to keep staging at 4KB.
                            sz = WSZ[wi]
                            w0 = WSTART[wi]
                            stk = wstg.tile([128, 8 * HD], BF16, tag="stk")
                            nc.sync.dma_start(
                                out=stk[:, 0:sz * HD].rearrange(
                                    "p (t c) -> p t c", t=sz),
                                in_=wk_d[w0 * 128:(w0 + sz) * 128, :]
                                    .rearrange("(t p) c -> p t c", p=128))
                            nc.vector.tensor_copy(
                                wk_t[wi][:], stk[:, 0:sz * HD])
                            stv = wstg.tile([128, 8 * HD], BF16, tag="stv")
                            nc.sync.dma_start(
                                out=stv[:, 0:sz * HD].rearrange(
                                    "p (t c) -> p t c", t=sz),
                                in_=wv_d[w0 * 128:(w0 + sz) * 128, :]
                                    .rearrange("(t p) c -> p t c", p=128))
                            nc.vector.tensor_copy(
                                wv_t[wi][:], stv[:, 0:sz * HD])
                            for hb in range(4 if sz > 3 else sz):
                                hs_ = max(1, sz // 4)
                                b0 = w0 + hb * hs_
                                if hb * hs_ >= sz:
                                    continue
                                stq = wstg.tile([128, 2 * G * HD], BF16,
                                                tag="stq")
                                nc.sync.dma_start(
                                    out=stq[:, 0:hs_ * G * HD].rearrange(
                                        "p (t c) -> p t c", t=hs_),
                                    in_=wq_d[b0 * 128:(b0 + hs_) * 128, :]
                                        .rearrange("(t p) c -> p t c", p=128))
                                nc.vector.tensor_copy(
                                    wq_t[wi][:, hb * hs_ * G * HD:
                                             (hb * hs_ + hs_) * G * HD],
                                    stq[:, 0:hs_ * G * HD])
                        hst = hpool.tile([128, QB], F32R, tag="hst")
                        nc.sync.dma_start(out=hst, in_=hsT_d[rows, cols])
                        if dt == 0:
                            nc.sync.dma_start(out=cosq_s, in_=cosq_d[:, cols])
                            nc.sync.dma_start(out=sinq_s, in_=sinq_d[:, cols])
                        st = dict(start=(dt == 0), stop=(dt == NDT - 1))
                        for h in range(G):
                            nc.tensor.matmul(
                                pq[h][:],
                                wq_t[wi][:, wo_ * G * HD + h * HD:
                                          wo_ * G * HD + (h + 1) * HD],
                                hst[:], **st)
                        nc.tensor.matmul(
                            pk[:], wk_t[wi][:, bass.ts(wo_, HD)], hst[:], **st)
                        nc.tensor.matmul(
                            pv[:], wv_t[wi][:, bass.ts(wo_, HD)], hst[:], **st)
                    # drain projection PSUM fast via ACT copies (frees the
                    # banks for the next qb ~0.6us after the last matmul),
                    # then rope lazily on DVE from the SBUF copies using the
                    # half-swapped sin table.
                    qsb = []
                    for h in range(G):
                        t = sbq.tile([128, QB], F32, tag=f"qsb{h}",
                                     name=f"qsb{h}")
                        nc.scalar.copy(t[:], pq[h][:])
                        qsb.append(t)
                    ksb = sbq.tile([128, QB], F32, tag="ksb")
                    nc.scalar.copy(ksb[:], pk[:])
                    vT_sb = scr.tile([128, QB], F32, tag="vT")
                    nc.scalar.copy(vT_sb[:], pv[:])
                    for j in range(QB // 128):
                        pst = ptps.tile([128, 128], F32, tag="pst")
                        nc.tensor.transpose(
                            pst[:], vT_sb[:, bass.ts(j, 128)], ident_s[:])
                        c = qb * (QB // 128) + j
                        nc.scalar.copy(v_s[:, bass.ts(c, 128)], pst[:])
                    for h in range(G):
                        t2 = scr.tile([128, QB], F32, tag="t2")
                        _rope_sb(nc, qT[h][:, cols], qsb[h][:], cosq_s[:],
                                 sinq_s[:], t2[:])
                    t2 = scr.tile([128, QB], F32, tag="t2")
                    _rope_sb(nc, kcurT[:, cols], ksb[:], cosq_s[:],
                             sinq_s[:], t2[:])
                    if qb in (0, 1):
                        # cache K/V load + rope, split across two blocks'
                        # DMA slack; DVE rope overlaps later projections.
                        _phase_a2(nc, tc, ktc_d, cosk_d, sink_d, ktc_r,
                                  vc_d, vc_s, a2p, qb)

            # ---------------- phase B: attention ---------------------------
            # aoT allocated here (not in res) so phase A can keep Wq resident.
            with tc.tile_pool(name="aob", bufs=1) as aob, \
                 tc.tile_pool(name="wost", bufs=1) as wopool:
                aoT = [aob.tile([128, Q], BF16, tag=f"aoT{h}", name=f"aoT{h}")
                       for h in range(G)]
                # all o_proj weights resident in bf16, loaded during phase B
                # (DMA is idle there); o_proj for qb<3 is woven into phase
                # B's kt loop to fill the PE bubbles of the exp-bound phase.
                wot = [[wopool.tile([128, QB], BF16, tag=f"wot{ht}_{dc}",
                                    name=f"wot{ht}_{dc}")
                        for dc in range(D // QB)] for ht in range(G)]
                for ht in range(G):
                    for dc in range(D // QB):
                        nc.sync.dma_start(
                            out=wot[ht][dc],
                            in_=wo_d[ht * 128:(ht + 1) * 128,
                                     dc * QB:(dc + 1) * QB])
                _phase_b(nc, tc, qT, kcurT, v_s, ktc_r, vc_s, maskb_s,
                         causal_s, onec_s, aoT, wot, out_d)
                # ---- tail o_proj: the last qb's blocks, 4-bank pipelined ---
                with tc.tile_pool(name="obt", bufs=4) as obuf2, \
                     tc.tile_pool(name="cpst", bufs=4, space="PSUM") as cps2:
                    for dc in range(D // QB):
                        for qt in range((NQB - 1) * (QB // 128), Q // 128):
                            pc = cps2.tile([128, QB], F32, tag="pc")
                            for ht in range(G):
                                nc.tensor.matmul(
                                    pc[:], aoT[ht][:, bass.ts(qt, 128)],
                                    wot[ht][dc][:],
                                    start=(ht == 0), stop=(ht == G - 1))
                            ob = obuf2.tile([128, QB], BF16, tag="ob")
                            if (dc + qt) % 2 == 0:
                                nc.scalar.copy(ob[:], pc[:])
                            else:
                                nc.vector.tensor_copy(ob[:], pc[:])
                            nc.sync.dma_start(
                                out=out_d[qt * 128:(qt + 1) * 128,
                                          dc * QB:(dc + 1) * QB],
                                in_=ob[:])
    _split_excess_waits(nc)
    return nc


_NC_CACHE = None


def _get_nc():
    global _NC_CACHE
    if _NC_CACHE is None:
        _NC_CACHE = build_nc()
    return _NC_CACHE


def _tables(pos):
    """cos/sin tables in [hd, n] layout; sin rows 0:63 negated (rope rot)."""
    inv_freq = 1.0 / (ROPE_BASE ** (np.arange(0, HD, 2, dtype=np.float32)
                                    / np.float32(HD)))
    inv_freq = inv_freq.astype(np.float32)
    ang = (pos.astype(np.float32)[None, :] * inv_freq[:, None]).astype(np.float32)
    a64 = ang.astype(np.float64)
    cos = np.cos(a64).astype(np.float32)
    sin = np.sin(a64).astype(np.float32)
    cosT = np.concatenate([cos, cos], axis=0)
    sinT = np.concatenate([-sin, sin], axis=0)
    return np.ascontiguousarray(cosT), np.ascontiguousarray(sinT)


def _prepare_in_maps(hidden_states, sink_k, sink_v, win_k, win_v, sink_pos,
                     key_pos, sink_mask, key_mask, Wq, Wk, Wv, Wo):
    hs = np.asarray(hidden_states, dtype=np.float32)[0]        # [Q, D]
    hsT = np.ascontiguousarray(hs.T)                            # [D, Q]
    Wq = np.asarray(Wq, dtype=np.float32)
    Wk = np.asarray(Wk, dtype=np.float32)
    Wv = np.asarray(Wv, dtype=np.float32)
    Wo = np.asarray(Wo, dtype=np.float32)
    sink_k = np.asarray(sink_k, dtype=np.float32)
    sink_v = np.asarray(sink_v, dtype=np.float32)
    win_k = np.asarray(win_k, dtype=np.float32)
    win_v = np.asarray(win_v, dtype=np.float32)
    spos = np.asarray(sink_pos).astype(np.int64)
    kpos = np.asarray(key_pos).astype(np.int64)
    smask = np.asarray(sink_mask, dtype=np.float32)
    kmask = np.asarray(key_mask, dtype=np.float32)

    max_pos = max(int(spos.max()), int(kpos.max())) + 1
    qpos = np.arange(Q, dtype=np.float64) + max_pos
    cosq, sinq = _tables(qpos)                                  # [128, Q]
    # q/k ropes read SBUF->SBUF: swap sin halves (rows 0:63 = +sin,
    # rows 64:127 = -sin) so DVE base partitions align
    sinq = np.ascontiguousarray(
        np.concatenate([-sinq[0:64], -sinq[64:128]], axis=0))
    cache_pos = np.concatenate([spos.astype(np.float64),
                                kpos.astype(np.float64),
                                np.zeros(KC - NS - NW)])
    cosk, sink_t = _tables(cache_pos)                           # [128, KC]
    # cache rope reads SBUF->SBUF: swap sin halves so base partitions align
    sink_t = np.ascontiguousarray(
        np.concatenate([-sink_t[0:64], -sink_t[64:128]], axis=0))

    maskb = np.concatenate([smask, kmask,
                            np.ones(KC - NS - NW, np.float32)]).astype(np.float32)
    maskb = maskb * np.float32(NEG)
    maskb_T = np.ascontiguousarray(maskb.reshape(NKC, 128).T)   # [128, NKC]

    causal01 = (np.arange(128)[:, None] <= np.arange(128)[None, :]) \
        .astype(np.float32)                                     # keep k<=q
    onec = np.ones((128, 1), np.float32)
    ident = np.eye(128, dtype=np.float32)

    Wq_h = Wq.reshape(D, H, HD)
    Wo_h = Wo.reshape(H, HD, D)
    pad = KC - NS - NW

    in_maps = []
    for c in range(NC_CORES):
        hsel = slice(c * G, (c + 1) * G)
        wq_c = np.ascontiguousarray(
            Wq_h[:, hsel].reshape(D, G * HD).astype(ml_dtypes.bfloat16))
        wk_c = np.ascontiguousarray(
            Wk[:, c * HD:(c + 1) * HD].astype(ml_dtypes.bfloat16))
        wv_c = np.ascontiguousarray(
            Wv[:, c * HD:(c + 1) * HD].astype(ml_dtypes.bfloat16))
        wo_c = np.ascontiguousarray(
            Wo_h[hsel].reshape(G * HD, D).astype(ml_dtypes.bfloat16))
        kc = np.concatenate([sink_k[0, c], win_k[0, c],
                             np.zeros((pad, HD), np.float32)], axis=0)  # [KC, HD]
        ktc = np.ascontiguousarray(kc.T)                                # [HD, KC]
        vc = np.concatenate([sink_v[0, c], win_v[0, c],
                             np.zeros((pad, HD), np.float32)], axis=0)
        in_maps.append(dict(
            hsT=hsT, wq=wq_c, wk=wk_c, wv=wv_c, wo=wo_c,
            ktc=ktc, vc=np.ascontiguousarray(vc),
            cosq=cosq, sinq=sinq, cosk=cosk, sink=sink_t,
            maskb=maskb_T, causal01=causal01,
            onec=onec, ident=ident,
        ))

    return in_maps


def kernel(**inputs):
    in_maps = _prepare_in_maps(**inputs)
    nc = _get_nc()
    res = run_bass_kernel_spmd(nc, in_maps, list(range(NC_CORES)))
    acc = np.zeros((Q, D), dtype=np.float64)
    for r in res.results:
        acc += np.asarray(r["out"], dtype=np.float64)
    return acc.astype(np.float32)[None]


if __name__ == "__main__":
    nc = build_nc()
    ni = sum(len(bb.instructions) for f in nc.m.functions for bb in f.blocks)
    print(f"built ok: {ni} instructions")
